# revision 9
# baseline (speedup 1.0000x reference)
"""Trainium2 Bass kernel for nn_Equilibrium (scatter_memory).

Computation (reference):
    x_out[t, 4m+0..3] = [x[t,0,m], x[t,2,m], x[t,2,m], x[t,1,m]]   # [T, 4M]
    f = einsum('ti,nio->nto', x_out, weight)                       # [N, T, 2]
    f_in = f[node_in - 1]                                          # [N_in, T, 2]
    f_b  = boundary sums over top/bottom/left/right                # [6, T, 1]

Strategy: only nodes referenced by node_in/top/bottom/left/right need f.
Host dedupes those indices (~5.7k unique of 10k), shards the unique set
across 8 cores, and pre-lays-out each core's weight rows as a transposed
[k', n] matrix so the contraction dim sits on SBUF partitions with fully
contiguous DMA.  The device streams its weight shard (memory-bound) through
TensorE:  f32 values are split into bf16 hi+lo parts on host; the PE
computes all four cross products at bf16 rate into fp32 PSUM, giving
~fp32 accuracy.  Host reassembles f_in (gather) and the 6 tiny boundary
sums from the per-core outputs.
"""

import os
import sys
import types

import numpy as np
import ml_dtypes

import concourse.bass as bass
import concourse.mybir as mybir
import concourse.tile as tile
from concourse.vector_clock import ScopedClock
from concourse.bass_utils import run_bass_kernel_spmd

N_CORES = 8
BF16 = ml_dtypes.bfloat16

# mode: "hilo" (bf16 hi/lo split, ~fp32 accurate), "f32", "f32r"
MODE = os.environ.get("EQ_KERNEL_MODE", "hilo")
CHUNKS_PER_DMA = int(os.environ.get("EQ_CHUNKS_PER_DMA", "4"))
TRACE = bool(int(os.environ.get("EQ_KERNEL_TRACE", "0")))
LAST_EXEC_NS = None
LAST_RESULTS = None

# ---------------------------------------------------------------------------
# Patch 1: the Tile tail drain may carry more sem waits than walrus's
# per-instruction cap (1 for CTRL/Drain in this toolchain). Split them.
_DRAIN_MAX_WAITS = 1


def _patched_drain_and_barrier(self, tick_clock, wait_clock):
    nc = self.nc
    drain_inst = nc.sync.drain()
    wait_clock.add_sem_waits(
        drain_inst.ins, ScopedClock({None: tick_clock.global_clock})
    )
    si = drain_inst.ins.sync_info
    if si is not None and len(si.on_wait) > _DRAIN_MAX_WAITS:
        waits = list(si.on_wait)
        si.on_wait = waits[:_DRAIN_MAX_WAITS]
        for i in range(_DRAIN_MAX_WAITS, len(waits), _DRAIN_MAX_WAITS):
            extra = nc.sync.drain()
            extra.ins.sync_info = mybir.SyncInfo(
                on_wait=list(waits[i : i + _DRAIN_MAX_WAITS]), on_update=[]
            )
    nc.all_engine_barrier()
    assert self.sems is not None
    popped = nc._tile_sem_poison_stack.pop()
    assert popped is self._sem_poison
    nc.clear_and_free_semaphores(list(self.sems.allocated().values()))
    nc.all_engine_barrier()


tile.TileContext._drain_and_barrier = _patched_drain_and_barrier


def _split_waits(nc, cap=_DRAIN_MAX_WAITS):
    """Walrus in this toolchain accepts at most one sem wait per
    instruction; hoist extra waits onto preceding same-engine NoOps."""
    n_id = 0
    for fn in nc.m.functions:
        for bb in fn.blocks:
            insts = list(bb.instructions)
            out = []
            changed = False
            for inst in insts:
                si = inst.sync_info
                if si is not None and len(si.on_wait) > cap:
                    waits = list(si.on_wait)
                    n_extra = len(waits) - cap
                    for i in range(0, n_extra, cap):
                        nop = mybir.InstNoOp(
                            name=f"waitsplit_{n_id}",
                            engine=inst.engine,
                            bass_nofuse=True,
                            sync_info=mybir.SyncInfo(
                                on_wait=list(waits[i : i + cap]), on_update=[]
                            ),
                        )
                        n_id += 1
                        nc.register_instruction(nop)
                        out.append(nop)
                    si.on_wait = waits[n_extra:]
                    changed = True
                out.append(inst)
            if changed:
                bb.instructions = out


# Patch 2: NTFF profile hook (only needed when tracing; the image's antenv
# lacks axon_hooks, so register a ctypes-based hook ourselves).
def _install_ntff_hook():
    try:
        from antenv.axon_hooks import get_axon_ntff_profile_hook  # noqa: F401

        return
    except ImportError:
        pass
    try:
        from trn_agent_boot.trn_boot import _ntff_profile_via_ctypes

        hook = _ntff_profile_via_ctypes("/opt/axon/libaxon_pjrt.so")
    except Exception:
        hook = None
    import antenv

    mod = types.ModuleType("antenv.axon_hooks")
    mod.get_axon_ntff_profile_hook = lambda: hook
    sys.modules["antenv.axon_hooks"] = mod
    antenv.axon_hooks = mod


# ---------------------------------------------------------------------------
# Device kernel builder


def _build_nc(Uc, mode, chunks_per_dma):
    """Build the per-core Bass program.

    Inputs (per core):
      w  : [G, 128, A*C]  moving operand (weight shard, transposed layout)
           G*A = 64 k-chunks of 128; C columns per chunk-row block.
      x2 : [128, 64*M]    stationary operand; chunk kb occupies cols kb*M..
    Output:
      f  : [32, Uc] f32   f[o*16+t, u]
    """
    K_CHUNKS = 64
    A = chunks_per_dma
    G = K_CHUNKS // A
    if mode == "hilo":
        M, s = 64, 2
        dt_in = mybir.dt.bfloat16
    elif mode == "f32":
        M, s = 32, 1
        dt_in = mybir.dt.float32
    elif mode == "f32r":
        M, s = 32, 1
        dt_in = mybir.dt.float32r
    else:
        raise ValueError(mode)
    C = s * Uc
    NB = 512
    n_blocks = (C + NB - 1) // NB

    nc = bass.Bass("TRN2", target_bir_lowering=False, debug=False,
                   num_devices=N_CORES)
    w = nc.dram_tensor("w", [G, 128, A * C], dt_in, kind="ExternalInput").ap()
    x2 = nc.dram_tensor("x2", [128, K_CHUNKS * M], dt_in,
                        kind="ExternalInput").ap()
    f = nc.dram_tensor("f", [32, Uc], mybir.dt.float32,
                       kind="ExternalOutput").ap()

    with tile.TileContext(nc) as tc:
        with (
            tc.tile_pool(name="x2p", bufs=1) as x2p,
            tc.tile_pool(name="wp", bufs=3) as wp,
            tc.tile_pool(name="psum", bufs=1, space="PSUM") as pp,
            tc.tile_pool(name="outp", bufs=1) as outp,
        ):
            x2_t = x2p.tile([128, K_CHUNKS * M], dt_in)
            nc.sync.dma_start(x2_t[:], x2[:])

            psums = [pp.tile([M, min(NB, C - nb * NB)], mybir.dt.float32,
                             name=f"ps{nb}")
                     for nb in range(n_blocks)]
            out_t = outp.tile([32, Uc], mybir.dt.float32)

            for g in range(G):
                w_t = wp.tile([128, A * C], dt_in, tag="w")
                nc.sync.dma_start(w_t[:], w[g, :, :])
                for a in range(A):
                    kb = g * A + a
                    lhsT = x2_t[:, kb * M : (kb + 1) * M]
                    for nb in range(n_blocks):
                        w_nb = min(NB, C - nb * NB)
                        rhs = w_t[:, a * C + nb * NB : a * C + nb * NB + w_nb]
                        nc.tensor.matmul(
                            psums[nb][:, :],
                            lhsT,
                            rhs,
                            start=(kb == 0),
                            stop=(kb == K_CHUNKS - 1),
                        )

            # Combine psum blocks into f32 output [32, Uc].
            for nb in range(n_blocks):
                w_nb = min(NB, C - nb * NB)
                if mode == "hilo":
                    # sum the x_hi/x_lo partition halves, then the
                    # interleaved W_hi/W_lo column pairs.
                    tmp = outp.tile([32, w_nb], mybir.dt.float32, tag="tmp")
                    nc.vector.tensor_copy(tmp[:, :], psums[nb][0:32, :])
                    nc.vector.tensor_add(tmp[:, :], tmp[:, :],
                                         psums[nb][32:64, :])
                    u0 = nb * NB // 2
                    nu = w_nb // 2
                    t3 = tmp.rearrange("p (u s) -> p u s", s=2)
                    nc.vector.tensor_add(
                        out_t[:, u0 : u0 + nu],
                        t3[:, :, 0],
                        t3[:, :, 1],
                    )
                else:
                    nc.vector.tensor_copy(out_t[:, nb * NB : nb * NB + w_nb],
                                          psums[nb][:, :])
            nc.sync.dma_start(f[:], out_t[:])
    _split_waits(nc)
    return nc


_NC_CACHE = {}


def _get_nc(Uc, mode, chunks_per_dma):
    key = (Uc, mode, chunks_per_dma)
    if key not in _NC_CACHE:
        _NC_CACHE[key] = _build_nc(Uc, mode, chunks_per_dma)
    return _NC_CACHE[key]


# ---------------------------------------------------------------------------
# Host side


def _split_hilo(arr_f32):
    hi = arr_f32.astype(BF16)
    lo = (arr_f32 - hi.astype(np.float32)).astype(BF16)
    return hi, lo


def _prepare(x, weight, node_in, top, bottom, left, right, mode, A):
    """Host prep: dedup indices, build per-core input maps. Returns
    (in_maps, meta)."""
    T, three, Mdim = x.shape
    assert three == 3
    N, K4, two = weight.shape
    K = K4  # 4*M

    # ---- dedup + shard -----------------------------------------------------
    idx_all = np.concatenate([node_in, top, bottom, left, right]) - 1
    uniq, inv = np.unique(idx_all, return_inverse=True)
    U0 = len(uniq)
    Uc = -(-U0 // N_CORES)
    Uc = ((Uc + 7) // 8) * 8  # pad to multiple of 8
    Upad = Uc * N_CORES
    uniq_pad = np.zeros(Upad, dtype=np.int64)
    uniq_pad[:U0] = uniq

    # ---- x_out and stationary operand -------------------------------------
    xo = np.stack([x[:, 0, :], x[:, 2, :], x[:, 2, :], x[:, 1, :]],
                  axis=-1).reshape(T, K)  # [T, 4M]
    K2 = 2 * K  # interleaved contraction k' = 2i + o
    K_CHUNKS = K2 // 128
    assert K2 == K_CHUNKS * 128

    if mode == "hilo":
        M = 64
        np_dt = BF16
        x_hi, x_lo = _split_hilo(xo)
        X2 = np.zeros((K2, M), dtype=np_dt)
        X2[0::2, 0:T] = x_hi.T
        X2[1::2, T : 2 * T] = x_hi.T
        X2[0::2, 2 * T : 3 * T] = x_lo.T
        X2[1::2, 3 * T : 4 * T] = x_lo.T
    else:
        M = 32
        np_dt = np.float32
        X2 = np.zeros((K2, M), dtype=np_dt)
        X2[0::2, 0:T] = xo.T
        X2[1::2, T : 2 * T] = xo.T
    x2r = np.ascontiguousarray(
        X2.reshape(K_CHUNKS, 128, M).transpose(1, 0, 2).reshape(128,
                                                                K_CHUNKS * M)
    )

    # ---- per-core weight shards -------------------------------------------
    wf = weight.reshape(N, K2)  # row n: k' = 2i+o contiguous
    G = K_CHUNKS // A
    in_maps = []
    for c in range(N_CORES):
        rows = uniq_pad[c * Uc : (c + 1) * Uc]
        Wg = wf[rows]  # [Uc, K2] f32
        if mode == "hilo":
            hi, lo = _split_hilo(Wg)
            V = np.empty((2 * Uc, K2), dtype=np_dt)
            V[0::2] = hi
            V[1::2] = lo
        else:
            V = Wg
        Cc = V.shape[0]  # s*Uc
        # [K2, Cc] with chunk-grouped layout [G, 128, A*Cc]
        wt = np.ascontiguousarray(
            V.T.reshape(G, A, 128, Cc).transpose(0, 2, 1, 3).reshape(
                G, 128, A * Cc
            )
        )
        in_maps.append({"w": wt, "x2": x2r})

    meta = dict(T=T, Uc=Uc, Upad=Upad, inv=inv, mode=mode, A=A)
    return in_maps, meta


def _assemble(per_core_f, meta, node_in, top):
    """Unshard: build f_uniq, gather f_in, boundary sums."""
    T, Uc, Upad, inv = meta["T"], meta["Uc"], meta["Upad"], meta["inv"]
    n_in = node_in.shape[0]
    n_b = top.shape[0]
    f_uniq = np.empty((Upad, T, 2), dtype=np.float32)
    for c in range(N_CORES):
        fc = per_core_f[c]  # [32, Uc]: rows o*T+t
        f_uniq[c * Uc : (c + 1) * Uc] = fc.reshape(2, T, Uc).transpose(2, 1, 0)

    f_in = np.ascontiguousarray(f_uniq[inv[:n_in]])
    bi = inv[n_in:]
    s_top = f_uniq[bi[0:n_b], :, 1].sum(axis=0)
    s_bot = f_uniq[bi[n_b : 2 * n_b], :, 1].sum(axis=0)
    s_left = f_uniq[bi[2 * n_b : 3 * n_b], :, 0].sum(axis=0)
    s_right = f_uniq[bi[3 * n_b : 4 * n_b], :, 0].sum(axis=0)
    f_b = np.stack(
        [s_top, s_bot, s_left, s_right, s_top + s_bot, s_left + s_right],
        axis=0,
    )[..., None].astype(np.float32)
    return f_in, f_b


def kernel(x, weight, node_in, top, bottom, left, right):
    global LAST_EXEC_NS, LAST_RESULTS
    x = np.asarray(x, dtype=np.float32)
    weight = np.asarray(weight, dtype=np.float32)
    node_in = np.asarray(node_in).astype(np.int64)
    top = np.asarray(top).astype(np.int64)
    bottom = np.asarray(bottom).astype(np.int64)
    left = np.asarray(left).astype(np.int64)
    right = np.asarray(right).astype(np.int64)

    in_maps, meta = _prepare(x, weight, node_in, top, bottom, left, right,
                             MODE, CHUNKS_PER_DMA)
    nc = _get_nc(meta["Uc"], MODE, CHUNKS_PER_DMA)
    if TRACE:
        _install_ntff_hook()
        import tempfile

        res = run_bass_kernel_spmd(
            nc, in_maps, list(range(N_CORES)), trace=True,
            tmpdir=tempfile.mkdtemp(prefix="eq_trace_"),
        )
        LAST_EXEC_NS = res.exec_time_ns
    else:
        res = run_bass_kernel_spmd(nc, in_maps, list(range(N_CORES)))
    LAST_RESULTS = res

    per_core_f = [res.results[c]["f"] for c in range(N_CORES)]
    return _assemble(per_core_f, meta, node_in, top)


# revision 12
# speedup vs baseline: 1.1796x; 1.1796x over previous
"""Trainium2 Bass kernel for nn_Equilibrium (scatter_memory).

Computation (reference):
    x_out[t, 4m+0..3] = [x[t,0,m], x[t,2,m], x[t,2,m], x[t,1,m]]   # [T, 4M]
    f = einsum('ti,nio->nto', x_out, weight)                       # [N, T, 2]
    f_in = f[node_in - 1]                                          # [N_in, T, 2]
    f_b  = boundary sums over top/bottom/left/right                # [6, T, 1]

Strategy: only nodes referenced by node_in/top/bottom/left/right need f.
Host dedupes those indices (~5.7k unique of 10k), shards the unique set
across 8 cores, and pre-lays-out each core's weight rows as a transposed
[k', n] matrix so the contraction dim sits on SBUF partitions with fully
contiguous DMA.  The device streams its weight shard (memory-bound) through
TensorE:  f32 values are split into bf16 hi+lo parts on host; the PE
computes all four cross products at bf16 rate into fp32 PSUM, giving
~fp32 accuracy.  Host reassembles f_in (gather) and the 6 tiny boundary
sums from the per-core outputs.
"""

import os
import sys
import types

import numpy as np
import ml_dtypes

import concourse.bass as bass
import concourse.mybir as mybir
import concourse.tile as tile
from concourse.vector_clock import ScopedClock
from concourse.bass_utils import run_bass_kernel_spmd

N_CORES = 8
BF16 = ml_dtypes.bfloat16

# mode: "hilo" (bf16 hi/lo split, ~fp32 accurate), "f32", "f32r"
MODE = os.environ.get("EQ_KERNEL_MODE", "hilo")
CHUNKS_PER_DMA = int(os.environ.get("EQ_CHUNKS_PER_DMA", "8"))
TRACE = bool(int(os.environ.get("EQ_KERNEL_TRACE", "0")))
LAST_EXEC_NS = None
LAST_RESULTS = None

# ---------------------------------------------------------------------------
# Patch 1: the Tile tail drain may carry more sem waits than walrus's
# per-instruction cap (1 for CTRL/Drain in this toolchain). Split them.
_DRAIN_MAX_WAITS = 1


def _patched_drain_and_barrier(self, tick_clock, wait_clock):
    nc = self.nc
    drain_inst = nc.sync.drain()
    wait_clock.add_sem_waits(
        drain_inst.ins, ScopedClock({None: tick_clock.global_clock})
    )
    si = drain_inst.ins.sync_info
    if si is not None and len(si.on_wait) > _DRAIN_MAX_WAITS:
        waits = list(si.on_wait)
        si.on_wait = waits[:_DRAIN_MAX_WAITS]
        for i in range(_DRAIN_MAX_WAITS, len(waits), _DRAIN_MAX_WAITS):
            extra = nc.sync.drain()
            extra.ins.sync_info = mybir.SyncInfo(
                on_wait=list(waits[i : i + _DRAIN_MAX_WAITS]), on_update=[]
            )
    nc.all_engine_barrier()
    assert self.sems is not None
    popped = nc._tile_sem_poison_stack.pop()
    assert popped is self._sem_poison
    nc.clear_and_free_semaphores(list(self.sems.allocated().values()))
    nc.all_engine_barrier()


tile.TileContext._drain_and_barrier = _patched_drain_and_barrier


def _split_waits(nc, cap=_DRAIN_MAX_WAITS):
    """Walrus in this toolchain accepts at most one sem wait per
    instruction; hoist extra waits onto preceding same-engine NoOps."""
    n_id = 0
    for fn in nc.m.functions:
        for bb in fn.blocks:
            insts = list(bb.instructions)
            out = []
            changed = False
            for inst in insts:
                si = inst.sync_info
                if si is not None and len(si.on_wait) > cap:
                    waits = list(si.on_wait)
                    n_extra = len(waits) - cap
                    for i in range(0, n_extra, cap):
                        nop = mybir.InstNoOp(
                            name=f"waitsplit_{n_id}",
                            engine=inst.engine,
                            bass_nofuse=True,
                            sync_info=mybir.SyncInfo(
                                on_wait=list(waits[i : i + cap]), on_update=[]
                            ),
                        )
                        n_id += 1
                        nc.register_instruction(nop)
                        out.append(nop)
                    si.on_wait = waits[n_extra:]
                    changed = True
                out.append(inst)
            if changed:
                bb.instructions = out


# Patch 2: NTFF profile hook (only needed when tracing; the image's antenv
# lacks axon_hooks, so register a ctypes-based hook ourselves).
def _install_ntff_hook():
    try:
        from antenv.axon_hooks import get_axon_ntff_profile_hook  # noqa: F401

        return
    except ImportError:
        pass
    try:
        from trn_agent_boot.trn_boot import _ntff_profile_via_ctypes

        hook = _ntff_profile_via_ctypes("/opt/axon/libaxon_pjrt.so")
    except Exception:
        hook = None
    import antenv

    mod = types.ModuleType("antenv.axon_hooks")
    mod.get_axon_ntff_profile_hook = lambda: hook
    sys.modules["antenv.axon_hooks"] = mod
    antenv.axon_hooks = mod


# ---------------------------------------------------------------------------
# Device kernel builder


def _build_nc(Uc, mode, chunks_per_dma, wp_bufs=4):
    """Build the per-core Bass program.

    hilo mode stacks W_hi then W_lo along the contraction axis (128
    k-chunks; the stationary x2 = [x_hi | x_lo] repeats for both halves),
    so PSUM accumulates all four bf16 cross products: psum[0:32] =
    x_hi@(W_hi+W_lo), psum[32:64] = x_lo@(W_hi+W_lo); one DVE add
    finishes the job.

    Inputs (per core):
      w  : [G, 128, A*Uc]  moving operand (weight shard, transposed)
           G*A = n_chunks k-chunks of 128 rows; Uc columns each.
      x2 : [128, 64*M]     stationary operand; chunk kb uses cols (kb%64)*M.
    Output:
      f  : [32, Uc] f32    f[o*16+t, u]
    """
    A = chunks_per_dma
    if mode == "hilo":
        M = 64
        n_chunks = 128
        dt_in = mybir.dt.bfloat16
    elif mode == "f32":
        M = 32
        n_chunks = 64
        dt_in = mybir.dt.float32
    elif mode == "f32r":
        M = 32
        n_chunks = 64
        dt_in = mybir.dt.float32r
    else:
        raise ValueError(mode)
    G = n_chunks // A
    C = Uc
    NB = 512
    n_blocks = (C + NB - 1) // NB

    nc = bass.Bass("TRN2", target_bir_lowering=False, debug=False,
                   num_devices=N_CORES)
    w = nc.dram_tensor("w", [G, 128, A * C], dt_in, kind="ExternalInput").ap()
    x2 = nc.dram_tensor("x2", [128, 64 * M], dt_in,
                        kind="ExternalInput").ap()
    f = nc.dram_tensor("f", [32, Uc], mybir.dt.float32,
                       kind="ExternalOutput").ap()

    with tile.TileContext(nc) as tc:
        with (
            tc.tile_pool(name="x2p", bufs=1) as x2p,
            tc.tile_pool(name="wp", bufs=wp_bufs) as wp,
            tc.tile_pool(name="psum", bufs=1, space="PSUM") as pp,
            tc.tile_pool(name="outp", bufs=1) as outp,
        ):
            x2_t = x2p.tile([128, 64 * M], dt_in)
            nc.sync.dma_start(x2_t[:], x2[:])

            psums = [pp.tile([M, min(NB, C - nb * NB)], mybir.dt.float32,
                             name=f"ps{nb}")
                     for nb in range(n_blocks)]
            out_t = outp.tile([32, Uc], mybir.dt.float32)

            for g in range(G):
                w_t = wp.tile([128, A * C], dt_in, tag="w")
                # alternate the two HWDGE queues (SP / ACT)
                eng = nc.sync if g % 2 == 0 else nc.scalar
                eng.dma_start(w_t[:], w[g, :, :])
                for a in range(A):
                    kb = g * A + a
                    j = kb % 64
                    lhsT = x2_t[:, j * M : (j + 1) * M]
                    for nb in range(n_blocks):
                        w_nb = min(NB, C - nb * NB)
                        rhs = w_t[:, a * C + nb * NB : a * C + nb * NB + w_nb]
                        nc.tensor.matmul(
                            psums[nb][:, :],
                            lhsT,
                            rhs,
                            start=(kb == 0),
                            stop=(kb == n_chunks - 1),
                        )

            # Combine psum blocks into f32 output [32, Uc].
            for nb in range(n_blocks):
                w_nb = min(NB, C - nb * NB)
                sl = slice(nb * NB, nb * NB + w_nb)
                if mode == "hilo":
                    nc.vector.tensor_copy(out_t[:, sl], psums[nb][0:32, :])
                    nc.vector.tensor_add(out_t[:, sl], out_t[:, sl],
                                         psums[nb][32:64, :])
                else:
                    nc.vector.tensor_copy(out_t[:, sl], psums[nb][:, :])
            nc.sync.dma_start(f[:], out_t[:])
    _split_waits(nc)
    return nc


_NC_CACHE = {}


def _get_nc(Uc, mode, chunks_per_dma):
    key = (Uc, mode, chunks_per_dma)
    if key not in _NC_CACHE:
        _NC_CACHE[key] = _build_nc(Uc, mode, chunks_per_dma)
    return _NC_CACHE[key]


# ---------------------------------------------------------------------------
# Host side


def _split_hilo(arr_f32):
    hi = arr_f32.astype(BF16)
    lo = (arr_f32 - hi.astype(np.float32)).astype(BF16)
    return hi, lo


def _prepare(x, weight, node_in, top, bottom, left, right, mode, A):
    """Host prep: dedup indices, build per-core input maps. Returns
    (in_maps, meta)."""
    T, three, Mdim = x.shape
    assert three == 3
    N, K4, two = weight.shape
    K = K4  # 4*M

    # ---- dedup + shard -----------------------------------------------------
    idx_all = np.concatenate([node_in, top, bottom, left, right]) - 1
    uniq, inv = np.unique(idx_all, return_inverse=True)
    U0 = len(uniq)
    Uc = -(-U0 // N_CORES)
    Uc = ((Uc + 7) // 8) * 8  # pad to multiple of 8
    Upad = Uc * N_CORES
    uniq_pad = np.zeros(Upad, dtype=np.int64)
    uniq_pad[:U0] = uniq

    # ---- x_out and stationary operand -------------------------------------
    xo = np.stack([x[:, 0, :], x[:, 2, :], x[:, 2, :], x[:, 1, :]],
                  axis=-1).reshape(T, K)  # [T, 4M]
    K2 = 2 * K  # interleaved contraction k' = 2i + o
    K_CHUNKS = K2 // 128
    assert K2 == K_CHUNKS * 128

    if mode == "hilo":
        M = 64
        np_dt = BF16
        x_hi, x_lo = _split_hilo(xo)
        X2 = np.zeros((K2, M), dtype=np_dt)
        X2[0::2, 0:T] = x_hi.T
        X2[1::2, T : 2 * T] = x_hi.T
        X2[0::2, 2 * T : 3 * T] = x_lo.T
        X2[1::2, 3 * T : 4 * T] = x_lo.T
    else:
        M = 32
        np_dt = np.float32
        X2 = np.zeros((K2, M), dtype=np_dt)
        X2[0::2, 0:T] = xo.T
        X2[1::2, T : 2 * T] = xo.T
    x2r = np.ascontiguousarray(
        X2.reshape(K_CHUNKS, 128, M).transpose(1, 0, 2).reshape(128,
                                                                K_CHUNKS * M)
    )

    # ---- per-core weight shards -------------------------------------------
    wf = weight.reshape(N, K2)  # row n: k' = 2i+o contiguous
    n_chunks = 2 * K_CHUNKS if mode == "hilo" else K_CHUNKS
    G = n_chunks // A
    in_maps = []
    for c in range(N_CORES):
        rows = uniq_pad[c * Uc : (c + 1) * Uc]
        Wg = wf[rows]  # [Uc, K2] f32
        if mode == "hilo":
            hi, lo = _split_hilo(Wg)
            V = np.concatenate([hi, lo], axis=1)  # [Uc, 2*K2]
        else:
            V = Wg
        Kt = V.shape[1]  # n_chunks*128
        # [Kt, Uc] with chunk-grouped layout [G, 128, A*Uc]
        wt = np.ascontiguousarray(
            V.T.reshape(G, A, 128, Uc).transpose(0, 2, 1, 3).reshape(
                G, 128, A * Uc
            )
        )
        in_maps.append({"w": wt, "x2": x2r})

    meta = dict(T=T, Uc=Uc, Upad=Upad, inv=inv, mode=mode, A=A)
    return in_maps, meta


def _assemble(per_core_f, meta, node_in, top):
    """Unshard: build f_uniq, gather f_in, boundary sums."""
    T, Uc, Upad, inv = meta["T"], meta["Uc"], meta["Upad"], meta["inv"]
    n_in = node_in.shape[0]
    n_b = top.shape[0]
    f_uniq = np.empty((Upad, T, 2), dtype=np.float32)
    for c in range(N_CORES):
        fc = per_core_f[c]  # [32, Uc]: rows o*T+t
        f_uniq[c * Uc : (c + 1) * Uc] = fc.reshape(2, T, Uc).transpose(2, 1, 0)

    f_in = np.ascontiguousarray(f_uniq[inv[:n_in]])
    bi = inv[n_in:]
    s_top = f_uniq[bi[0:n_b], :, 1].sum(axis=0)
    s_bot = f_uniq[bi[n_b : 2 * n_b], :, 1].sum(axis=0)
    s_left = f_uniq[bi[2 * n_b : 3 * n_b], :, 0].sum(axis=0)
    s_right = f_uniq[bi[3 * n_b : 4 * n_b], :, 0].sum(axis=0)
    f_b = np.stack(
        [s_top, s_bot, s_left, s_right, s_top + s_bot, s_left + s_right],
        axis=0,
    )[..., None].astype(np.float32)
    return f_in, f_b


def kernel(x, weight, node_in, top, bottom, left, right):
    global LAST_EXEC_NS, LAST_RESULTS
    x = np.asarray(x, dtype=np.float32)
    weight = np.asarray(weight, dtype=np.float32)
    node_in = np.asarray(node_in).astype(np.int64)
    top = np.asarray(top).astype(np.int64)
    bottom = np.asarray(bottom).astype(np.int64)
    left = np.asarray(left).astype(np.int64)
    right = np.asarray(right).astype(np.int64)

    in_maps, meta = _prepare(x, weight, node_in, top, bottom, left, right,
                             MODE, CHUNKS_PER_DMA)
    nc = _get_nc(meta["Uc"], MODE, CHUNKS_PER_DMA)
    if TRACE:
        _install_ntff_hook()
        import tempfile

        res = run_bass_kernel_spmd(
            nc, in_maps, list(range(N_CORES)), trace=True,
            tmpdir=tempfile.mkdtemp(prefix="eq_trace_"),
        )
        LAST_EXEC_NS = res.exec_time_ns
    else:
        res = run_bass_kernel_spmd(nc, in_maps, list(range(N_CORES)))
    LAST_RESULTS = res

    per_core_f = [res.results[c]["f"] for c in range(N_CORES)]
    return _assemble(per_core_f, meta, node_in, top)


# revision 16
# speedup vs baseline: 1.2072x; 1.0234x over previous
"""Trainium2 Bass kernel for nn_Equilibrium (scatter_memory).

Computation (reference):
    x_out[t, 4m+0..3] = [x[t,0,m], x[t,2,m], x[t,2,m], x[t,1,m]]   # [T, 4M]
    f = einsum('ti,nio->nto', x_out, weight)                       # [N, T, 2]
    f_in = f[node_in - 1]                                          # [N_in, T, 2]
    f_b  = boundary sums over top/bottom/left/right                # [6, T, 1]

Strategy: only nodes referenced by node_in/top/bottom/left/right need f.
Host dedupes those indices (~5.7k unique of 10k), shards the unique set
across 8 cores, and pre-lays-out each core's weight rows as a transposed
[k', n] matrix so the contraction dim sits on SBUF partitions with fully
contiguous DMA.  The device streams its weight shard (memory-bound) through
TensorE:  f32 values are split into bf16 hi+lo parts on host; the PE
computes all four cross products at bf16 rate into fp32 PSUM, giving
~fp32 accuracy.  Host reassembles f_in (gather) and the 6 tiny boundary
sums from the per-core outputs.
"""

import os
import sys
import types

import numpy as np
import ml_dtypes

import concourse.bass as bass
import concourse.mybir as mybir
import concourse.tile as tile
from concourse.vector_clock import ScopedClock
from concourse.bass_utils import run_bass_kernel_spmd

N_CORES = 8
BF16 = ml_dtypes.bfloat16

# mode: "hilo" (bf16 hi/lo split, ~fp32 accurate), "f32", "f32r"
MODE = os.environ.get("EQ_KERNEL_MODE", "hilo")
CHUNKS_PER_DMA = int(os.environ.get("EQ_CHUNKS_PER_DMA", "8"))
TRACE = bool(int(os.environ.get("EQ_KERNEL_TRACE", "0")))
LAST_EXEC_NS = None
LAST_RESULTS = None

# ---------------------------------------------------------------------------
# Patch 1: the Tile tail drain may carry more sem waits than walrus's
# per-instruction cap (1 for CTRL/Drain in this toolchain). Split them.
_DRAIN_MAX_WAITS = 1


def _patched_drain_and_barrier(self, tick_clock, wait_clock):
    nc = self.nc
    drain_inst = nc.sync.drain()
    wait_clock.add_sem_waits(
        drain_inst.ins, ScopedClock({None: tick_clock.global_clock})
    )
    si = drain_inst.ins.sync_info
    if si is not None and len(si.on_wait) > _DRAIN_MAX_WAITS:
        waits = list(si.on_wait)
        si.on_wait = waits[:_DRAIN_MAX_WAITS]
        for i in range(_DRAIN_MAX_WAITS, len(waits), _DRAIN_MAX_WAITS):
            extra = nc.sync.drain()
            extra.ins.sync_info = mybir.SyncInfo(
                on_wait=list(waits[i : i + _DRAIN_MAX_WAITS]), on_update=[]
            )
    nc.all_engine_barrier()
    assert self.sems is not None
    popped = nc._tile_sem_poison_stack.pop()
    assert popped is self._sem_poison
    nc.clear_and_free_semaphores(list(self.sems.allocated().values()))
    nc.all_engine_barrier()


tile.TileContext._drain_and_barrier = _patched_drain_and_barrier


def _split_waits(nc, cap=_DRAIN_MAX_WAITS):
    """Walrus in this toolchain accepts at most one sem wait per
    instruction; hoist extra waits onto preceding same-engine NoOps."""
    n_id = 0
    for fn in nc.m.functions:
        for bb in fn.blocks:
            insts = list(bb.instructions)
            out = []
            changed = False
            for inst in insts:
                si = inst.sync_info
                if si is not None and len(si.on_wait) > cap:
                    waits = list(si.on_wait)
                    n_extra = len(waits) - cap
                    for i in range(0, n_extra, cap):
                        nop = mybir.InstNoOp(
                            name=f"waitsplit_{n_id}",
                            engine=inst.engine,
                            bass_nofuse=True,
                            sync_info=mybir.SyncInfo(
                                on_wait=list(waits[i : i + cap]), on_update=[]
                            ),
                        )
                        n_id += 1
                        nc.register_instruction(nop)
                        out.append(nop)
                    si.on_wait = waits[n_extra:]
                    changed = True
                out.append(inst)
            if changed:
                bb.instructions = out


# Patch 2: NTFF profile hook (only needed when tracing; the image's antenv
# lacks axon_hooks, so register a ctypes-based hook ourselves).
def _install_ntff_hook():
    try:
        from antenv.axon_hooks import get_axon_ntff_profile_hook  # noqa: F401

        return
    except ImportError:
        pass
    try:
        from trn_agent_boot.trn_boot import _ntff_profile_via_ctypes

        hook = _ntff_profile_via_ctypes("/opt/axon/libaxon_pjrt.so")
    except Exception:
        hook = None
    import antenv

    mod = types.ModuleType("antenv.axon_hooks")
    mod.get_axon_ntff_profile_hook = lambda: hook
    sys.modules["antenv.axon_hooks"] = mod
    antenv.axon_hooks = mod


# ---------------------------------------------------------------------------
# Device kernel builder


def _build_nc(Uc, mode, chunks_per_dma, wp_bufs=4):
    """Build the per-core Bass program.

    hilo mode stacks W_hi then W_lo along the contraction axis (128
    k-chunks; the stationary x2 = [x_hi | x_lo] repeats for both halves),
    so PSUM accumulates all four bf16 cross products: psum[0:32] =
    x_hi@(W_hi+W_lo), psum[32:64] = x_lo@(W_hi+W_lo); one DVE add
    finishes the job.

    Inputs (per core):
      w  : [G, 128, A*Uc]  moving operand (weight shard, transposed)
           G*A = n_chunks k-chunks of 128 rows; Uc columns each.
      x2 : [128, 64*M]     stationary operand; chunk kb uses cols (kb%64)*M.
    Output:
      f  : [32, Uc] f32    f[o*16+t, u]
    """
    A = chunks_per_dma
    if mode == "hilo":
        M = 64
        n_chunks = 128
        dt_in = mybir.dt.bfloat16
    elif mode == "f32":
        M = 32
        n_chunks = 64
        dt_in = mybir.dt.float32
    elif mode == "f32r":
        M = 32
        n_chunks = 64
        dt_in = mybir.dt.float32r
    elif mode == "fp16":
        M = 32
        n_chunks = 64
        dt_in = mybir.dt.float16
    else:
        raise ValueError(mode)
    G = n_chunks // A
    C = Uc
    NB = 512
    n_blocks = (C + NB - 1) // NB

    nc = bass.Bass("TRN2", target_bir_lowering=False, debug=False,
                   num_devices=N_CORES)
    w = nc.dram_tensor("w", [G, 128, A * C], dt_in, kind="ExternalInput").ap()
    x2 = nc.dram_tensor("x2", [128, 64 * M], dt_in,
                        kind="ExternalInput").ap()
    f = nc.dram_tensor("f", [32, Uc], mybir.dt.float32,
                       kind="ExternalOutput").ap()

    with tile.TileContext(nc) as tc:
        with (
            tc.tile_pool(name="x2p", bufs=1) as x2p,
            tc.tile_pool(name="wp", bufs=wp_bufs) as wp,
            tc.tile_pool(name="psum", bufs=1, space="PSUM") as pp,
            tc.tile_pool(name="outp", bufs=1) as outp,
        ):
            x2_t = x2p.tile([128, 64 * M], dt_in)
            nc.sync.dma_start(x2_t[:], x2[:])

            psums = [pp.tile([M, min(NB, C - nb * NB)], mybir.dt.float32,
                             name=f"ps{nb}")
                     for nb in range(n_blocks)]
            out_t = outp.tile([32, Uc], mybir.dt.float32)

            for g in range(G):
                w_t = wp.tile([128, A * C], dt_in, tag="w")
                # alternate the two HWDGE queues (ACT / SP); w0 goes on ACT
                # so it transfers concurrently with x2 (which is on SP).
                eng = nc.scalar if g % 2 == 0 else nc.sync
                eng.dma_start(w_t[:], w[g, :, :])
                for a in range(A):
                    kb = g * A + a
                    j = kb % 64
                    lhsT = x2_t[:, j * M : (j + 1) * M]
                    for nb in range(n_blocks):
                        w_nb = min(NB, C - nb * NB)
                        rhs = w_t[:, a * C + nb * NB : a * C + nb * NB + w_nb]
                        nc.tensor.matmul(
                            psums[nb][:, :],
                            lhsT,
                            rhs,
                            start=(kb == 0),
                            stop=(kb == n_chunks - 1),
                        )

            # Combine psum blocks into f32 output [32, Uc].
            for nb in range(n_blocks):
                w_nb = min(NB, C - nb * NB)
                sl = slice(nb * NB, nb * NB + w_nb)
                if mode == "hilo":
                    nc.vector.tensor_copy(out_t[:, sl], psums[nb][0:32, :])
                    nc.vector.tensor_add(out_t[:, sl], out_t[:, sl],
                                         psums[nb][32:64, :])
                else:
                    nc.vector.tensor_copy(out_t[:, sl], psums[nb][:, :])
            nc.sync.dma_start(f[:], out_t[:])
    _split_waits(nc)
    return nc


_NC_CACHE = {}


def _get_nc(Uc, mode, chunks_per_dma):
    key = (Uc, mode, chunks_per_dma)
    if key not in _NC_CACHE:
        _NC_CACHE[key] = _build_nc(Uc, mode, chunks_per_dma)
    return _NC_CACHE[key]


# ---------------------------------------------------------------------------
# Host side


def _split_hilo(arr_f32):
    hi = arr_f32.astype(BF16)
    lo = (arr_f32 - hi.astype(np.float32)).astype(BF16)
    return hi, lo


def _prepare(x, weight, node_in, top, bottom, left, right, mode, A):
    """Host prep: dedup indices, build per-core input maps. Returns
    (in_maps, meta)."""
    T, three, Mdim = x.shape
    assert three == 3
    N, K4, two = weight.shape
    K = K4  # 4*M

    # ---- dedup + shard -----------------------------------------------------
    idx_all = np.concatenate([node_in, top, bottom, left, right]) - 1
    uniq, inv = np.unique(idx_all, return_inverse=True)
    U0 = len(uniq)
    Uc = -(-U0 // N_CORES)
    Uc = ((Uc + 7) // 8) * 8  # pad to multiple of 8
    Upad = Uc * N_CORES
    uniq_pad = np.zeros(Upad, dtype=np.int64)
    uniq_pad[:U0] = uniq

    # ---- x_out and stationary operand -------------------------------------
    xo = np.stack([x[:, 0, :], x[:, 2, :], x[:, 2, :], x[:, 1, :]],
                  axis=-1).reshape(T, K)  # [T, 4M]
    K2 = 2 * K  # interleaved contraction k' = 2i + o
    K_CHUNKS = K2 // 128
    assert K2 == K_CHUNKS * 128

    if mode == "hilo":
        M = 64
        np_dt = BF16
        x_hi, x_lo = _split_hilo(xo)
        X2 = np.zeros((K2, M), dtype=np_dt)
        X2[0::2, 0:T] = x_hi.T
        X2[1::2, T : 2 * T] = x_hi.T
        X2[0::2, 2 * T : 3 * T] = x_lo.T
        X2[1::2, 3 * T : 4 * T] = x_lo.T
    else:
        M = 32
        np_dt = np.float16 if mode == "fp16" else np.float32
        X2 = np.zeros((K2, M), dtype=np_dt)
        X2[0::2, 0:T] = xo.T
        X2[1::2, T : 2 * T] = xo.T
    x2r = np.ascontiguousarray(
        X2.reshape(K_CHUNKS, 128, M).transpose(1, 0, 2).reshape(128,
                                                                K_CHUNKS * M)
    )

    # ---- per-core weight shards -------------------------------------------
    wf = weight.reshape(N, K2)  # row n: k' = 2i+o contiguous
    n_chunks = 2 * K_CHUNKS if mode == "hilo" else K_CHUNKS
    G = n_chunks // A
    in_maps = []
    for c in range(N_CORES):
        rows = uniq_pad[c * Uc : (c + 1) * Uc]
        Wg = wf[rows]  # [Uc, K2] f32
        if mode == "hilo":
            hi, lo = _split_hilo(Wg)
            V = np.concatenate([hi, lo], axis=1)  # [Uc, 2*K2]
        elif mode == "fp16":
            V = Wg.astype(np.float16)
        else:
            V = Wg
        Kt = V.shape[1]  # n_chunks*128
        # [Kt, Uc] with chunk-grouped layout [G, 128, A*Uc]
        wt = np.ascontiguousarray(
            V.T.reshape(G, A, 128, Uc).transpose(0, 2, 1, 3).reshape(
                G, 128, A * Uc
            )
        )
        in_maps.append({"w": wt, "x2": x2r})

    meta = dict(T=T, Uc=Uc, Upad=Upad, inv=inv, mode=mode, A=A)
    return in_maps, meta


def _assemble(per_core_f, meta, node_in, top):
    """Unshard: build f_uniq, gather f_in, boundary sums."""
    T, Uc, Upad, inv = meta["T"], meta["Uc"], meta["Upad"], meta["inv"]
    n_in = node_in.shape[0]
    n_b = top.shape[0]
    f_uniq = np.empty((Upad, T, 2), dtype=np.float32)
    for c in range(N_CORES):
        fc = per_core_f[c]  # [32, Uc]: rows o*T+t
        f_uniq[c * Uc : (c + 1) * Uc] = fc.reshape(2, T, Uc).transpose(2, 1, 0)

    f_in = np.ascontiguousarray(f_uniq[inv[:n_in]])
    bi = inv[n_in:]
    s_top = f_uniq[bi[0:n_b], :, 1].sum(axis=0)
    s_bot = f_uniq[bi[n_b : 2 * n_b], :, 1].sum(axis=0)
    s_left = f_uniq[bi[2 * n_b : 3 * n_b], :, 0].sum(axis=0)
    s_right = f_uniq[bi[3 * n_b : 4 * n_b], :, 0].sum(axis=0)
    f_b = np.stack(
        [s_top, s_bot, s_left, s_right, s_top + s_bot, s_left + s_right],
        axis=0,
    )[..., None].astype(np.float32)
    return f_in, f_b


def kernel(x, weight, node_in, top, bottom, left, right):
    global LAST_EXEC_NS, LAST_RESULTS
    x = np.asarray(x, dtype=np.float32)
    weight = np.asarray(weight, dtype=np.float32)
    node_in = np.asarray(node_in).astype(np.int64)
    top = np.asarray(top).astype(np.int64)
    bottom = np.asarray(bottom).astype(np.int64)
    left = np.asarray(left).astype(np.int64)
    right = np.asarray(right).astype(np.int64)

    in_maps, meta = _prepare(x, weight, node_in, top, bottom, left, right,
                             MODE, CHUNKS_PER_DMA)
    nc = _get_nc(meta["Uc"], MODE, CHUNKS_PER_DMA)
    if TRACE:
        _install_ntff_hook()
        import tempfile

        res = run_bass_kernel_spmd(
            nc, in_maps, list(range(N_CORES)), trace=True,
            tmpdir=tempfile.mkdtemp(prefix="eq_trace_"),
        )
        LAST_EXEC_NS = res.exec_time_ns
    else:
        res = run_bass_kernel_spmd(nc, in_maps, list(range(N_CORES)))
    LAST_RESULTS = res

    per_core_f = [res.results[c]["f"] for c in range(N_CORES)]
    return _assemble(per_core_f, meta, node_in, top)
